# revision 32
# baseline (speedup 1.0000x reference)
"""CenterLoss kernel v2 for Trainium2 (raw Bass), 8-core data-parallel, fp16.

Math: the reference's masked-distmat loss reduces to

    loss = ( sum_b clip(||x_b - centers[labels_b]||^2, 1e-12, 1e12)
             + (B*C - B) * 1e-12 ) / B

so each core gathers its 512 label rows and computes per-row squared
distances; the host does the final clip + tiny reduction.

v2 changes vs the 8521ns baseline:
  - fp16 on-device compute (host converts x/centers once).  The harness
    gate is rel_err < 2e-2; fp16 distances land ~1e-5 off the fp32 value.
    fp16 center rows are 1KB, so each of the four indirect gathers hits
    the SWDGE 500ns descriptor floor instead of 790ns -> the Pool gather
    wall shrinks from 3760ns to 2600ns.
  - engine schedule is self-clocked: DMA-completion semaphores observed
    by a waiter that is already blocked cost +1717/+1883ns (DGE wake
    latency), while a wait that arrives after the increment is free.
    DVE pads with disjoint junk memsets so each sub's waits arrive just
    after the gather commit.  Semaphores still carry all correctness.
  - per-tile pipeline: DVE fp16 subtract (327ns, 2x mode) for tiles 0-2;
    ACT Square+accum for tiles 0,1 (its table warmup reads a framework
    const AP so it starts at engine-start); tile 2 squared+reduced on DVE
    as TT-mult (327, 2x) + tensor_scalar accumulate (194, 4x) — cheaper
    than scalar_tensor_tensor (594).  TensorTensor is the only compute op
    walrus encodes on Pool; TensorScalarPtr/TensorTensorReduce are
    rejected there.
  - tile 3 (the last gather) is a three-engine bucket brigade: Pool
    subtracts it in two column chunks right after its last gather, then
    squares the back chunk while DVE squares the front chunk; DVE's final
    4x reduce lands ~790ns after the gather wall.  The split ACOL
    balances DVE-free time against Pool's chain to within ~20ns.
  - output is split: SP ships tiles 0-2 early; ACT ships tile 3's column
    the moment its accum commits (ACT self-clock pad), so the program's
    tail is a single minimal DMA + its fixed completion latency.
  - no on-device clip: host clips the 4096 per-row sums exactly.
"""

import os
import time
from contextlib import ExitStack

import numpy as np

# recover wedged NeuronCores left by a previous crashed run (pitfalls.md)
os.environ.setdefault("NEURON_RT_RESET_CORES", "1")

import concourse.bass as bass
import concourse.mybir as mybir
from concourse.bass_utils import run_bass_kernel_spmd

P = 128
B, C, D = 4096, 10000, 512
N_CORES = 8
ROWS = B // N_CORES   # 512 rows per core
NT = ROWS // P        # 4 tiles of 128 rows
CLAMP_MIN = 1e-12
CLAMP_MAX = 1e12

F16 = mybir.dt.float16
F32 = mybir.dt.float32

# self-clock pads (fp32 junk elems per memset); tuned against CoreSim
PAD0 = 817    # DVE: start -> arrive just after g0 commit (~1100)
PAD1 = 108    # DVE: after sub0 -> arrive just after g1 commit (~1600)
PAD2 = 108    # DVE: after sub1 -> arrive just after g2 commit (~2100)
APAD = 34     # ACT: pad activation [P, APAD] before the out DMA
ACOL = 336    # tile-3 column split: DVE squares [0:ACOL], Pool [ACOL:]

_cached_nc = None


def _build():
    nc = bass.Bass()
    x16 = nc.dram_tensor("x16", [ROWS, D], F16, kind="ExternalInput")
    lab32 = nc.dram_tensor("lab32", [P, NT], mybir.dt.int32, kind="ExternalInput")
    cen16 = nc.dram_tensor("cen16", [C, D], F16, kind="ExternalInput")
    out_a = nc.dram_tensor("out_a", [P, NT - 1], F32, kind="ExternalOutput")
    out_b = nc.dram_tensor("out_b", [P, 1], F32, kind="ExternalOutput")

    with ExitStack() as ctx:
        lab = ctx.enter_context(nc.sbuf_tensor("lab", [P, NT], mybir.dt.int32))
        xt = ctx.enter_context(nc.sbuf_tensor("xt", [P, NT, D], F16))
        ct = ctx.enter_context(nc.sbuf_tensor("ct", [P, NT, D], F16))
        diff = ctx.enter_context(nc.sbuf_tensor("diff", [P, NT, D], F16))
        sq = ctx.enter_context(nc.sbuf_tensor("sq", [P, NT, D], F16))
        acc = ctx.enter_context(nc.sbuf_tensor("acc", [P, NT], F32))
        junk = ctx.enter_context(nc.sbuf_tensor("junk", [P, 2048], F32))
        junk2 = ctx.enter_context(nc.sbuf_tensor("junk2", [P, D], F16))
        junk3 = ctx.enter_context(nc.sbuf_tensor("junk3", [P, D], F16))
        zb = ctx.enter_context(nc.sbuf_tensor("zb", [P, 1], F16))
        wu = ctx.enter_context(nc.sbuf_tensor("wu", [P, 1], F16))
        apad = ctx.enter_context(nc.sbuf_tensor("apad", [P, APAD], F16))

        slab = ctx.enter_context(nc.semaphore("slab"))
        sc = [ctx.enter_context(nc.semaphore(f"sc{t}")) for t in range(NT)]
        sx = [ctx.enter_context(nc.semaphore(f"sx{i}")) for i in range(2)]
        dv = ctx.enter_context(nc.semaphore("dv"))    # DVE: zb=1, subs=2..5
        ps = ctx.enter_context(nc.semaphore("ps"))    # Pool sub3 done
        so = ctx.enter_context(nc.semaphore("so"))    # out DMA
        asq = ctx.enter_context(nc.semaphore("asq"))  # ACT sq ops
        block = ctx.enter_context(nc.Block())

        @block.gpsimd
        def _(g):
            # labels self-loaded so the first gather sees them at the DMA's
            # engine-end (same-engine observation skips the DGE wake latency)
            g.dma_start(out=lab[:], in_=lab32[:]).then_inc(slab, 16)
            g.wait_ge(slab, 16)
            for t in range(NT):
                # HW DGE only honors [P, 1] offset APs (a [P, NT] offset AP
                # gathers garbage on HW despite simulating correctly)
                g.indirect_dma_start(
                    out=ct[:, t, :],
                    out_offset=None,
                    in_=cen16[:],
                    in_offset=bass.IndirectOffsetOnAxis(ap=lab[:, t:t + 1], axis=0),
                ).then_inc(sc[t], 16)
            # tile 3's subtract right after the gather stream (TensorTensor is
            # the only compute op walrus encodes on Pool); both waits arrive
            # after their commits (x23 at 1780, own g3 at engine end), so free.
            # Done in halves so DVE can start squaring the first half while
            # Pool still subtracts the second.
            g.wait_ge(sx[1], 16)
            g.wait_ge(sc[3], 16)
            nc.gpsimd.tensor_tensor(
                out=diff[:, 3, 0:ACOL], in0=xt[:, 3, 0:ACOL],
                in1=ct[:, 3, 0:ACOL], op=mybir.AluOpType.subtract,
            ).then_inc(ps, 1)
            nc.gpsimd.tensor_tensor(
                out=diff[:, 3, ACOL:], in0=xt[:, 3, ACOL:],
                in1=ct[:, 3, ACOL:], op=mybir.AluOpType.subtract,
            ).then_inc(ps, 1)
            # Pool also squares the back half of tile 3 in parallel with DVE
            # squaring the front half; the split is balanced so both finish
            # just before DVE's final reduce wants them
            g.wait_ge(ps, 2)
            nc.gpsimd.tensor_tensor(
                out=sq[:, 3, ACOL:], in0=diff[:, 3, ACOL:],
                in1=diff[:, 3, ACOL:], op=mybir.AluOpType.mult,
            ).then_inc(ps, 1)

        @block.sync
        def _(sync):
            for i in range(2):
                src = x16[i * 2 * P:(i + 1) * 2 * P, :].rearrange(
                    "(j p) d -> p j d", j=2, p=P
                )
                sync.dma_start(out=xt[:, 2 * i:2 * i + 2, :], in_=src).then_inc(sx[i], 16)
            # tiles 0-2 shipped early (off the critical path); the final DMA
            # then only gates on tile 3's accum
            sync.wait_ge(asq, 2)
            sync.wait_ge(dv, 6)
            sync.dma_start(out=out_a[:], in_=acc[:, 0:3]).then_inc(so, 16)

        @block.vector
        def _(vector):
            nc.vector.memset(zb[:], 0.0).then_inc(dv, 1)
            nc.vector.memset(junk[:, 0:PAD0], 0.0)
            off = PAD0
            for t in range(3):
                if t == 0:
                    vector.wait_ge(sx[0], 16)
                if t == 2:
                    vector.wait_ge(sx[1], 16)
                vector.wait_ge(sc[t], 16)
                nc.vector.tensor_tensor(
                    out=diff[:, t, :], in0=xt[:, t, :], in1=ct[:, t, :],
                    op=mybir.AluOpType.subtract,
                ).then_inc(dv, 1)
                if t < 2:
                    pad = (PAD1, PAD2)[t]
                    nc.vector.memset(junk[:, off:off + pad], 0.0)
                    off += pad
            # square+rowsum for tiles 2,3 as fp16 TT-mult (327, 2x mode) +
            # tensor_scalar accumulate (194, 4x mode) — cheaper than the
            # 594ns scalar_tensor_tensor.  Tile 3's diff halves come from
            # Pool; every wait here arrives after its commit, so all free.
            vector.wait_ge(dv, 4)
            nc.vector.tensor_tensor(
                out=sq[:, 2, :], in0=diff[:, 2, :], in1=diff[:, 2, :],
                op=mybir.AluOpType.mult,
            ).then_inc(dv, 1)
            vector.wait_ge(dv, 5)
            nc.vector.tensor_scalar(
                junk2[:], sq[:, 2, :], 0.0, None,
                mybir.AluOpType.add, mybir.AluOpType.add, acc[:, 2:3],
            ).then_inc(dv, 1)
            vector.wait_ge(ps, 1)
            nc.vector.tensor_tensor(
                out=sq[:, 3, 0:ACOL], in0=diff[:, 3, 0:ACOL],
                in1=diff[:, 3, 0:ACOL], op=mybir.AluOpType.mult,
            ).then_inc(dv, 1)
            vector.wait_ge(dv, 7)
            vector.wait_ge(ps, 3)
            nc.vector.tensor_scalar(
                junk3[:], sq[:, 3, :], 0.0, None,
                mybir.AluOpType.add, mybir.AluOpType.add, acc[:, 3:4],
            ).then_inc(dv, 1)

        @block.scalar
        def _(scalar):
            # warm the Square activation table during the DMA window; input is
            # a framework-preamble const AP so no cross-engine wait is needed
            nc.scalar.activation(
                out=wu[:, :1], in_=nc.const_aps.tensor(0.0, (P, 1), F32),
                func=mybir.ActivationFunctionType.Square, bias=0.0, scale=1.0,
            )
            scalar.wait_ge(dv, 2)
            nc.scalar.activation(
                out=sq[:, 0, :], in_=diff[:, 0, :],
                func=mybir.ActivationFunctionType.Square, bias=0.0, scale=1.0,
                accum_out=acc[:, 0:1],
            ).then_inc(asq, 1)
            scalar.wait_ge(dv, 3)
            nc.scalar.activation(
                out=sq[:, 1, :], in_=diff[:, 1, :],
                func=mybir.ActivationFunctionType.Square, bias=0.0, scale=1.0,
                accum_out=acc[:, 1:2],
            ).then_inc(asq, 1)
            if APAD:
                # self-clock pad: arrive at the dv wait just after DVE's
                # last accum commits (reads diff0, already covered by dv>=2)
                nc.scalar.activation(
                    out=apad[:], in_=diff[:, 0, 0:APAD],
                    func=mybir.ActivationFunctionType.Square, bias=0.0, scale=1.0,
                )
            scalar.wait_ge(dv, 8)
            scalar.dma_start(out=out_b[:], in_=acc[:, 3:4]).then_inc(so, 16)

    return nc


def _prep_labels32(labels: np.ndarray) -> np.ndarray:
    """int32 [128, NT] with [p, t] = labels[t*128 + p]."""
    return np.ascontiguousarray(labels.astype(np.int32).reshape(NT, P).T)


def _run(inputs, trace=False):
    global _cached_nc
    if _cached_nc is None:
        _cached_nc = _build()
    nc = _cached_nc

    x16 = np.ascontiguousarray(np.asarray(inputs["x"], dtype=np.float32).astype(np.float16))
    labels = np.asarray(inputs["labels"])
    cen16 = np.ascontiguousarray(
        np.asarray(inputs["centers"], dtype=np.float32).astype(np.float16))

    in_maps = []
    for c in range(N_CORES):
        sl = slice(c * ROWS, (c + 1) * ROWS)
        in_maps.append({
            "x16": x16[sl],
            "lab32": _prep_labels32(labels[sl]),
            "cen16": cen16,
        })
    last_err = None
    for attempt in range(4):  # transient NRT exec errors recover on retry
        try:
            res = run_bass_kernel_spmd(nc, in_maps, list(range(N_CORES)), trace=trace)
            break
        except Exception as e:  # noqa: BLE001
            last_err = e
            # a wedged NeuronCore (NRT_EXEC_UNIT_UNRECOVERABLE) survives
            # in-process retries unless the PJRT client is rebuilt with
            # NEURON_RT_RESET_CORES=1 in effect
            os.environ["NEURON_RT_RESET_CORES"] = "1"
            try:
                import jax

                jax.clear_caches()
                jax.extend.backend.clear_backends()
            except Exception:  # noqa: BLE001
                pass
            time.sleep(1 + 2 * attempt)
    else:
        raise last_err
    partials = np.stack([
        np.concatenate([res.results[i]["out_a"], res.results[i]["out_b"]], axis=1)
        for i in range(N_CORES)
    ])
    clipped = np.clip(partials.astype(np.float64), CLAMP_MIN, CLAMP_MAX)
    loss = clipped.sum() / B + (C - 1) * CLAMP_MIN
    return np.float32(loss), res


def kernel(**inputs) -> np.ndarray:
    val, _ = _run(inputs, trace=False)
    return np.asarray(val, dtype=np.float32)


# revision 34
# speedup vs baseline: 1.0019x; 1.0019x over previous
"""CenterLoss kernel v2 for Trainium2 (raw Bass), 8-core data-parallel, fp16.

Math: the reference's masked-distmat loss reduces to

    loss = ( sum_b clip(||x_b - centers[labels_b]||^2, 1e-12, 1e12)
             + (B*C - B) * 1e-12 ) / B

so each core gathers its 512 label rows and computes per-row squared
distances; the host does the final clip + tiny reduction.

v2 changes vs the 8521ns baseline:
  - fp16 on-device compute (host converts x/centers once).  The harness
    gate is rel_err < 2e-2; fp16 distances land ~1e-5 off the fp32 value.
    fp16 center rows are 1KB, so each of the four indirect gathers hits
    the SWDGE 500ns descriptor floor instead of 790ns -> the Pool gather
    wall shrinks from 3760ns to 2600ns.
  - engine schedule is self-clocked: DMA-completion semaphores observed
    by a waiter that is already blocked cost +1717/+1883ns (DGE wake
    latency), while a wait that arrives after the increment is free.
    DVE pads with disjoint junk memsets so each sub's waits arrive just
    after the gather commit.  Semaphores still carry all correctness.
  - per-tile pipeline: DVE fp16 subtract (327ns, 2x mode) for tiles 0-2;
    ACT Square+accum for tiles 0,1 (its table warmup reads a framework
    const AP so it starts at engine-start); tile 2 squared+reduced on DVE
    as TT-mult (327, 2x) + tensor_scalar accumulate (194, 4x) — cheaper
    than scalar_tensor_tensor (594).  TensorTensor is the only compute op
    walrus encodes on Pool; TensorScalarPtr/TensorTensorReduce are
    rejected there.
  - tile 3 (the last gather) is a three-engine bucket brigade: Pool
    subtracts it in two column chunks right after its last gather, then
    squares the back chunk while DVE squares the front chunk; DVE's final
    4x reduce lands ~790ns after the gather wall.  The split ACOL
    balances DVE-free time against Pool's chain to within ~20ns.
  - output is split: SP ships tiles 0-2 early; ACT ships tile 3's column
    the moment its accum commits (ACT self-clock pad), so the program's
    tail is a single minimal DMA + its fixed completion latency.
  - no on-device clip: host clips the 4096 per-row sums exactly.
"""

import os
import time
from contextlib import ExitStack

import numpy as np

# recover wedged NeuronCores left by a previous crashed run (pitfalls.md)
os.environ.setdefault("NEURON_RT_RESET_CORES", "1")

import concourse.bass as bass
import concourse.mybir as mybir
from concourse.bass_utils import run_bass_kernel_spmd

P = 128
B, C, D = 4096, 10000, 512
N_CORES = 8
ROWS = B // N_CORES   # 512 rows per core
NT = ROWS // P        # 4 tiles of 128 rows
CLAMP_MIN = 1e-12
CLAMP_MAX = 1e12

F16 = mybir.dt.float16
F32 = mybir.dt.float32

# self-clock pads (fp32 junk elems per memset); tuned against CoreSim
PAD0 = 817    # DVE: start -> arrive just after g0 commit (~1100)
PAD1 = 108    # DVE: after sub0 -> arrive just after g1 commit (~1600)
PAD2 = 101    # DVE: after sub1 -> arrive just after g2 commit (~2100)
APAD = 21     # ACT: pad activation [P, APAD] before the out DMA
ACOL = 336    # tile-3 column split: DVE squares [0:ACOL], Pool [ACOL:]

_cached_nc = None


def _build():
    nc = bass.Bass()
    x16 = nc.dram_tensor("x16", [ROWS, D], F16, kind="ExternalInput")
    lab32 = nc.dram_tensor("lab32", [P, NT], mybir.dt.int32, kind="ExternalInput")
    cen16 = nc.dram_tensor("cen16", [C, D], F16, kind="ExternalInput")
    out_a = nc.dram_tensor("out_a", [P, NT - 1], F32, kind="ExternalOutput")
    out_b = nc.dram_tensor("out_b", [P, 1], F32, kind="ExternalOutput")

    with ExitStack() as ctx:
        lab = ctx.enter_context(nc.sbuf_tensor("lab", [P, NT], mybir.dt.int32))
        xt = ctx.enter_context(nc.sbuf_tensor("xt", [P, NT, D], F16))
        ct = ctx.enter_context(nc.sbuf_tensor("ct", [P, NT, D], F16))
        diff = ctx.enter_context(nc.sbuf_tensor("diff", [P, NT, D], F16))
        sq = ctx.enter_context(nc.sbuf_tensor("sq", [P, NT, D], F16))
        acc = ctx.enter_context(nc.sbuf_tensor("acc", [P, NT], F32))
        junk = ctx.enter_context(nc.sbuf_tensor("junk", [P, 2048], F32))
        junk2 = ctx.enter_context(nc.sbuf_tensor("junk2", [P, D], F16))
        junk3 = ctx.enter_context(nc.sbuf_tensor("junk3", [P, D], F16))
        zb = ctx.enter_context(nc.sbuf_tensor("zb", [P, 1], F16))
        wu = ctx.enter_context(nc.sbuf_tensor("wu", [P, 1], F16))
        apad = ctx.enter_context(nc.sbuf_tensor("apad", [P, APAD], F16))

        slab = ctx.enter_context(nc.semaphore("slab"))
        sc = [ctx.enter_context(nc.semaphore(f"sc{t}")) for t in range(NT)]
        sx = [ctx.enter_context(nc.semaphore(f"sx{i}")) for i in range(2)]
        dv = ctx.enter_context(nc.semaphore("dv"))    # DVE: zb=1, subs=2..5
        ps = ctx.enter_context(nc.semaphore("ps"))    # Pool sub3 done
        so = ctx.enter_context(nc.semaphore("so"))    # out DMA
        asq = ctx.enter_context(nc.semaphore("asq"))  # ACT sq ops
        block = ctx.enter_context(nc.Block())

        @block.gpsimd
        def _(g):
            # labels self-loaded so the first gather sees them at the DMA's
            # engine-end (same-engine observation skips the DGE wake latency)
            g.dma_start(out=lab[:], in_=lab32[:]).then_inc(slab, 16)
            g.wait_ge(slab, 16)
            for t in range(NT):
                # HW DGE only honors [P, 1] offset APs (a [P, NT] offset AP
                # gathers garbage on HW despite simulating correctly)
                g.indirect_dma_start(
                    out=ct[:, t, :],
                    out_offset=None,
                    in_=cen16[:],
                    in_offset=bass.IndirectOffsetOnAxis(ap=lab[:, t:t + 1], axis=0),
                ).then_inc(sc[t], 16)
            # tile 3's subtract right after the gather stream (TensorTensor is
            # the only compute op walrus encodes on Pool); both waits arrive
            # after their commits (x23 at 1780, own g3 at engine end), so free.
            # Done in halves so DVE can start squaring the first half while
            # Pool still subtracts the second.
            g.wait_ge(sx[1], 16)
            g.wait_ge(sc[3], 16)
            nc.gpsimd.tensor_tensor(
                out=diff[:, 3, 0:ACOL], in0=xt[:, 3, 0:ACOL],
                in1=ct[:, 3, 0:ACOL], op=mybir.AluOpType.subtract,
            ).then_inc(ps, 1)
            nc.gpsimd.tensor_tensor(
                out=diff[:, 3, ACOL:], in0=xt[:, 3, ACOL:],
                in1=ct[:, 3, ACOL:], op=mybir.AluOpType.subtract,
            ).then_inc(ps, 1)
            # Pool also squares the back half of tile 3 in parallel with DVE
            # squaring the front half; the split is balanced so both finish
            # just before DVE's final reduce wants them
            g.wait_ge(ps, 2)
            nc.gpsimd.tensor_tensor(
                out=sq[:, 3, ACOL:], in0=diff[:, 3, ACOL:],
                in1=diff[:, 3, ACOL:], op=mybir.AluOpType.mult,
            ).then_inc(ps, 1)

        @block.sync
        def _(sync):
            for i in range(2):
                src = x16[i * 2 * P:(i + 1) * 2 * P, :].rearrange(
                    "(j p) d -> p j d", j=2, p=P
                )
                sync.dma_start(out=xt[:, 2 * i:2 * i + 2, :], in_=src).then_inc(sx[i], 16)
            # tiles 0-2 shipped early (off the critical path); the final DMA
            # then only gates on tile 3's accum
            sync.wait_ge(asq, 2)
            sync.wait_ge(dv, 6)
            sync.dma_start(out=out_a[:], in_=acc[:, 0:3]).then_inc(so, 16)

        @block.vector
        def _(vector):
            nc.vector.memset(zb[:], 0.0).then_inc(dv, 1)
            nc.vector.memset(junk[:, 0:PAD0], 0.0)
            off = PAD0
            for t in range(3):
                if t == 0:
                    vector.wait_ge(sx[0], 16)
                if t == 2:
                    vector.wait_ge(sx[1], 16)
                vector.wait_ge(sc[t], 16)
                nc.vector.tensor_tensor(
                    out=diff[:, t, :], in0=xt[:, t, :], in1=ct[:, t, :],
                    op=mybir.AluOpType.subtract,
                ).then_inc(dv, 1)
                if t < 2:
                    pad = (PAD1, PAD2)[t]
                    nc.vector.memset(junk[:, off:off + pad], 0.0)
                    off += pad
            # square+rowsum for tiles 2,3 as fp16 TT-mult (327, 2x mode) +
            # tensor_scalar accumulate (194, 4x mode) — cheaper than the
            # 594ns scalar_tensor_tensor.  Tile 3's diff halves come from
            # Pool; every wait here arrives after its commit, so all free.
            vector.wait_ge(dv, 4)
            nc.vector.tensor_tensor(
                out=sq[:, 2, :], in0=diff[:, 2, :], in1=diff[:, 2, :],
                op=mybir.AluOpType.mult,
            ).then_inc(dv, 1)
            vector.wait_ge(dv, 5)
            nc.vector.tensor_scalar(
                junk2[:], sq[:, 2, :], 0.0, None,
                mybir.AluOpType.add, mybir.AluOpType.add, acc[:, 2:3],
            ).then_inc(dv, 1)
            vector.wait_ge(ps, 1)
            nc.vector.tensor_tensor(
                out=sq[:, 3, 0:ACOL], in0=diff[:, 3, 0:ACOL],
                in1=diff[:, 3, 0:ACOL], op=mybir.AluOpType.mult,
            ).then_inc(dv, 1)
            vector.wait_ge(dv, 7)
            vector.wait_ge(ps, 3)
            nc.vector.tensor_scalar(
                junk3[:], sq[:, 3, :], 0.0, None,
                mybir.AluOpType.add, mybir.AluOpType.add, acc[:, 3:4],
            ).then_inc(dv, 1)

        @block.scalar
        def _(scalar):
            # warm the Square activation table during the DMA window; input is
            # a framework-preamble const AP so no cross-engine wait is needed
            nc.scalar.activation(
                out=wu[:, :1], in_=nc.const_aps.tensor(0.0, (P, 1), F32),
                func=mybir.ActivationFunctionType.Square, bias=0.0, scale=1.0,
            )
            scalar.wait_ge(dv, 2)
            nc.scalar.activation(
                out=sq[:, 0, :], in_=diff[:, 0, :],
                func=mybir.ActivationFunctionType.Square, bias=0.0, scale=1.0,
                accum_out=acc[:, 0:1],
            ).then_inc(asq, 1)
            scalar.wait_ge(dv, 3)
            nc.scalar.activation(
                out=sq[:, 1, :], in_=diff[:, 1, :],
                func=mybir.ActivationFunctionType.Square, bias=0.0, scale=1.0,
                accum_out=acc[:, 1:2],
            ).then_inc(asq, 1)
            if APAD:
                # self-clock pad: arrive at the dv wait just after DVE's
                # last accum commits (reads diff0, already covered by dv>=2)
                nc.scalar.activation(
                    out=apad[:], in_=diff[:, 0, 0:APAD],
                    func=mybir.ActivationFunctionType.Square, bias=0.0, scale=1.0,
                )
            scalar.wait_ge(dv, 8)
            scalar.dma_start(out=out_b[:], in_=acc[:, 3:4]).then_inc(so, 16)

    return nc


def _prep_labels32(labels: np.ndarray) -> np.ndarray:
    """int32 [128, NT] with [p, t] = labels[t*128 + p]."""
    return np.ascontiguousarray(labels.astype(np.int32).reshape(NT, P).T)


def _run(inputs, trace=False):
    global _cached_nc
    if _cached_nc is None:
        _cached_nc = _build()
    nc = _cached_nc

    x16 = np.ascontiguousarray(np.asarray(inputs["x"], dtype=np.float32).astype(np.float16))
    labels = np.asarray(inputs["labels"])
    cen16 = np.ascontiguousarray(
        np.asarray(inputs["centers"], dtype=np.float32).astype(np.float16))

    in_maps = []
    for c in range(N_CORES):
        sl = slice(c * ROWS, (c + 1) * ROWS)
        in_maps.append({
            "x16": x16[sl],
            "lab32": _prep_labels32(labels[sl]),
            "cen16": cen16,
        })
    last_err = None
    for attempt in range(4):  # transient NRT exec errors recover on retry
        try:
            res = run_bass_kernel_spmd(nc, in_maps, list(range(N_CORES)), trace=trace)
            break
        except Exception as e:  # noqa: BLE001
            last_err = e
            # a wedged NeuronCore (NRT_EXEC_UNIT_UNRECOVERABLE) survives
            # in-process retries unless the PJRT client is rebuilt with
            # NEURON_RT_RESET_CORES=1 in effect
            os.environ["NEURON_RT_RESET_CORES"] = "1"
            try:
                import jax

                jax.clear_caches()
                jax.extend.backend.clear_backends()
            except Exception:  # noqa: BLE001
                pass
            time.sleep(1 + 2 * attempt)
    else:
        raise last_err
    partials = np.stack([
        np.concatenate([res.results[i]["out_a"], res.results[i]["out_b"]], axis=1)
        for i in range(N_CORES)
    ])
    clipped = np.clip(partials.astype(np.float64), CLAMP_MIN, CLAMP_MAX)
    loss = clipped.sum() / B + (C - 1) * CLAMP_MIN
    return np.float32(loss), res


def kernel(**inputs) -> np.ndarray:
    val, _ = _run(inputs, trace=False)
    return np.asarray(val, dtype=np.float32)
